# revision 14
# baseline (speedup 1.0000x reference)
"""Trainium2 Bass kernel for CheemsMambaMixer (Mamba-1 selective SSM mixer).

Shapes: B=1, L=2048, H=1024, DI=2048, DS=16, DTR=64, K=4.
Sharding: tensor-parallel over the d_inner channel dim (256 channels/core on
8 cores).  The only cross-core communication is a [96, 2048] fp32 AllReduce
of the x_proj partial products; the out_proj row-parallel partials are summed
on the host.

Everything device-side runs in fp16 storage with fp32 accumulation (PSUM,
scan state), which lands ~1e-3 relative error vs the fp32 reference.
"""
import sys

sys.path.insert(0, "/opt/trn_rl_repo")

import numpy as np

import concourse.bass as bass
import concourse.tile as tile
from concourse import mybir
from concourse.bass_utils import run_bass_kernel_spmd
from concourse.tile_rust import add_dep_helper
import bass_rust as _bass_rust

# ---------------------------------------------------------------- constants
N_CORES = 8
B, L, H = 1, 2048, 1024
DI, DS, DTR, K = 2048, 16, 64, 4
DIL = DI // N_CORES          # 256 channels per core
NDT = DIL // 128             # 2 d-tiles of 128 channels
LC = 512                     # time chunk
NCH = L // LC                # 4 chunks
NTILE = DIL * DS // 128      # 32 (d,n)-tiles per core, 8 d x 16 n each
TPG = NTILE // NDT           # 16 tiles per d-group

F16 = mybir.dt.float16
F32 = mybir.dt.float32

N_PROCS = 27


class _SplitDrainTileContext(tile.TileContext):
    """Tail drain split into single-wait drains: the CTRL_NO ISA struct holds
    one sync-wait, but a kernel using all 8 HWDGE queues plus a collective
    accumulates 9+ outstanding procs at the tail."""

    def _drain_and_barrier(self, tick_clock, wait_clock):
        full = tick_clock.global_clock
        ticks = [(i, full.peek_next(i) - 1) for i in range(N_PROCS)]
        ticks = [(i, v) for i, v in ticks if v > 0]
        for i, v in ticks:
            c = _bass_rust.VectorClock()
            c.require_at_least(i, v)
            drain_inst = self.nc.sync.drain(fusable=False)
            wait_clock.add_sem_waits(
                drain_inst.ins, _bass_rust.ScopedClock({None: c}))
        self.nc.all_engine_barrier()
        assert self.sems is not None
        popped = self.nc._tile_sem_poison_stack.pop()
        assert popped is self._sem_poison
        self.nc.clear_and_free_semaphores(list(self.sems.allocated().values()))
        self.nc.all_engine_barrier()


def _split_multi_waits(nc):
    """TPB ISA structs carry a single sync-wait slot; Tile sometimes attaches
    several.  Hoist all but the last wait of every instruction onto dedicated
    single-wait NoOps on the same engine, inserted just before it."""
    wid = 0
    for bb in nc.main_func.blocks:
        insts = list(bb.instructions)
        out = []
        changed = False
        for ins in insts:
            si = ins.sync_info
            if si is not None and si.on_wait and len(si.on_wait) > 1:
                waits = list(si.on_wait)
                for w in waits[:-1]:
                    nop = _bass_rust.InstNoOp(name=f"W-split-{wid}", ins=[],
                                              outs=[])
                    wid += 1
                    nop.engine = ins.engine
                    nop.sync_info = mybir.SyncInfo(on_wait=[w], on_update=[])
                    out.append(nop)
                ins.sync_info = mybir.SyncInfo(on_wait=[waits[-1]],
                                               on_update=list(si.on_update or []))
                changed = True
            out.append(ins)
        if changed:
            bb.instructions = out


# ---------------------------------------------------------------- builder
def _build():
    nc = bass.Bass("TRN2", target_bir_lowering=False, debug=False,
                   num_devices=N_CORES)
    Act = mybir.ActivationFunctionType
    Op = mybir.AluOpType

    def din(name, shape, dtype=F16):
        return nc.dram_tensor(name, shape, dtype, kind="ExternalInput").ap()

    hsT = din("hsT", [H, L])                       # hidden_states[0].T
    wxzT = din("wxzT", [H, 2 * DIL])               # in_proj rows (x|z).T slice
    owT = din("owT", [DIL, H])                     # out_proj.T slice
    xpwT = din("xpwT", [DIL, 96])                  # x_proj.T slice
    dtwT = din("dtwT", [DTR, DIL])                 # dt_proj.T slice
    selrep = din("selrep", [128, TPG, 128])        # SelRep[k, li, p]
    selmap = din("selmap", [128, TPG, 128])        # Selmap[p, li, m]
    selbc = din("selbc", [96, 2, 128])             # SelB / SelC
    acols = din("acols", [128, NTILE], F32)        # A[d,n] per (tile, partition)
    convw = din("convw", [128, NDT, K], F32)
    convb = din("convb", [128, NDT], F32)
    dtb = din("dtb", [128, NDT], F32)
    dcol = din("dcol", [128, NDT], F32)
    out = nc.dram_tensor("out", [L, H], F32, kind="ExternalOutput").ap()

    with _SplitDrainTileContext(nc) as tc:
        import contextlib
        stack = contextlib.ExitStack()
        with stack:
            wpool = stack.enter_context(tc.tile_pool(name="wpool", bufs=1))
            state = stack.enter_context(tc.tile_pool(name="state", bufs=1))
            work = stack.enter_context(tc.tile_pool(name="work", bufs=3))
            psum = stack.enter_context(
                tc.tile_pool(name="psum", bufs=2, space="PSUM"))
            dram = stack.enter_context(
                tc.tile_pool(name="dram", bufs=1, space="DRAM"))

            # ---------------- load weights/constants
            wxzT_sb = wpool.tile([128, H // 128, 2 * DIL], F16)
            nc.sync.dma_start(wxzT_sb, wxzT.rearrange("(k p) m -> p k m", p=128))
            owT_sb = wpool.tile([128, NDT, H], F16)
            nc.sync.dma_start(owT_sb, owT.rearrange("(k p) h -> p k h", p=128))
            xpwT_sb = wpool.tile([128, NDT, 96], F16)
            nc.sync.dma_start(xpwT_sb, xpwT.rearrange("(k p) j -> p k j", p=128))
            dtwT_sb = wpool.tile([DTR, NDT, 128], F16)
            nc.sync.dma_start(dtwT_sb, dtwT.rearrange("k (m p) -> k m p", p=128))
            selrep_sb = wpool.tile([128, TPG, 128], F16)
            nc.sync.dma_start(selrep_sb, selrep)
            selmap_sb = wpool.tile([128, TPG, 128], F16)
            nc.sync.dma_start(selmap_sb, selmap)
            selbc_sb = wpool.tile([96, 2, 128], F16)
            nc.sync.dma_start(selbc_sb, selbc)
            acols_sb = wpool.tile([128, NTILE], F32)
            nc.sync.dma_start(acols_sb, acols)
            convw_sb = wpool.tile([128, NDT, K], F32)
            convw_dma = nc.sync.dma_start(convw_sb, convw)
            convb_sb = wpool.tile([128, NDT], F32)
            nc.sync.dma_start(convb_sb, convb)
            dtb_sb = wpool.tile([128, NDT], F32)
            nc.sync.dma_start(dtb_sb, dtb)
            dcol_sb = wpool.tile([128, NDT], F32)
            nc.sync.dma_start(dcol_sb, dcol)

            # Wait-slot fencing: TensorScalarPtr-class DVE ops (tensor_scalar,
            # scalar_tensor_tensor, tensor_tensor_scan) have very few sync-wait
            # slots in their ISA structs.  A tiny TensorTensor op (2 wait
            # slots) placed just before makes the in-order DVE observe the
            # producers' semaphores so the fragile op needs no new waits.
            fence_scratch = wpool.tile([128, 4], F32)

            def dve_observe(*insts):
                insts = [i for i in insts if i is not None]
                for j in range(0, len(insts), 1):
                    f = nc.vector.tensor_mul(
                        fence_scratch[:, 0:1], fence_scratch[:, 0:1],
                        fence_scratch[:, 0:1])
                    for d in insts[j:j + 1]:
                        add_dep_helper(f.ins, d.ins, sync=True,
                                       reason="dve wait fence")

            # The ACT engine loads ONE spline-table set; none contains
            # exp+silu+softplus together.  natural_log_exp_and_others has
            # {exp, ln, copy, identity}, so silu and softplus are synthesized:
            #   softplus(x) = ln(1 + e^x)
            #   silu(v) = v * sigma(v),  sigma(v) = exp(-ln(1 + e^-v))
            def silu_into(dst, v):
                t1 = work.tile([128, L], F32, tag="silu_t1", name="t1", bufs=1)
                nc.scalar.activation(t1, v, Act.Exp, scale=-1.0)
                nc.gpsimd.tensor_scalar_add(t1, t1, 1.0)
                t2 = work.tile([128, L], F32, tag="silu_t2", name="t2", bufs=1)
                nc.scalar.activation(t2, t1, Act.Ln)
                t3 = work.tile([128, L], F16, tag="silu_t3", name="t3", bufs=1)
                nc.scalar.activation(t3, t2, Act.Exp, scale=-1.0)
                nc.vector.tensor_mul(dst, v, t3)

            # persistent state tensors
            xc = [state.tile([128, L], F16, name=f"xc{i}") for i in range(NDT)]
            zsb = [state.tile([128, L], F16, name=f"zsb{i}") for i in range(NDT)]
            dt = [state.tile([128, L], F16, name=f"dt{i}") for i in range(NDT)]
            dtx = [state.tile([128, L], F16, name=f"dtx{i}") for i in range(NDT)]
            ssm_sb = state.tile([96, L], F32, name="ssm_sb")
            ssmr16 = state.tile([96, L], F16, name="ssmr16")

            # ---------------- phase 1: in_proj + conv + silu
            with tc.tile_pool(name="inproj", bufs=1) as inproj:
                hsT_sb = inproj.tile([128, H // 128, L], F16)
                nc.sync.dma_start(hsT_sb, hsT.rearrange("(k p) t -> p k t", p=128))
                xpad = [inproj.tile([128, K - 1 + L], F16, name=f"xpad{i}")
                        for i in range(NDT)]
                xpad_evacs = [[] for _ in range(NDT)]
                for i in range(NDT):
                    nc.vector.memset(xpad[i][:, 0:K - 1], 0.0)

                for dm in range(2 * NDT):
                    for tch in range(NCH):
                        ps = psum.tile([128, LC], F32, tag="mm")
                        for k in range(H // 128):
                            nc.tensor.matmul(
                                ps,
                                lhsT=wxzT_sb[:, k, 128 * dm:128 * (dm + 1)],
                                rhs=hsT_sb[:, k, LC * tch:LC * (tch + 1)],
                                start=(k == 0), stop=(k == H // 128 - 1))
                        if dm < NDT:  # x branch -> conv input
                            ev = nc.scalar.copy(
                                xpad[dm][:, K - 1 + LC * tch:K - 1 + LC * (tch + 1)],
                                ps)
                            xpad_evacs[dm].append(ev)
                        else:         # z branch -> SBUF, silu later
                            nc.scalar.copy(
                                zsb[dm - NDT][:, LC * tch:LC * (tch + 1)], ps)

                for i in range(NDT):
                    silu_into(zsb[i], zsb[i])

                # causal depthwise conv (K=4) + bias + silu
                for i in range(NDT):
                    acc = work.tile([128, L], F16, tag="convacc", bufs=2)
                    dve_observe(*xpad_evacs[i], convw_dma)
                    nc.vector.tensor_scalar(
                        acc, xpad[i][:, 0:L], convw_sb[:, i, 0:1], None,
                        op0=Op.mult)
                    for k in range(1, K):
                        nc.vector.scalar_tensor_tensor(
                            acc, xpad[i][:, k:k + L], convw_sb[:, i, k:k + 1],
                            acc, op0=Op.mult, op1=Op.add)
                    nc.vector.tensor_scalar_add(acc, acc, convb_sb[:, i:i + 1])
                    silu_into(xc[i], acc)

            # scan-phase persistents allocated after inproj released its zone
            scanp = stack.enter_context(tc.tile_pool(name="scanp", bufs=1))
            brep = scanp.tile([128, L], F16, name="brep")
            crep = scanp.tile([128, L], F16, name="crep")
            hbuf = [scanp.tile([128, TPG, LC], F16, name=f"hbuf{g}")
                    for g in range(NDT)]
            ysb = [scanp.tile([128, L], F16, name=f"ysb{g}") for g in range(NDT)]
            yg = [scanp.tile([128, L], F16, name=f"yg{g}") for g in range(NDT)]

            # ---------------- phase 2: x_proj partial + AllReduce
            for tch in range(NCH):
                ps = psum.tile([128, LC], F32, tag="mm", name="ssm_ps")
                for ki in range(NDT):
                    nc.tensor.matmul(
                        ps[0:96, :], lhsT=xpwT_sb[:, ki, :],
                        rhs=xc[ki][:, LC * tch:LC * (tch + 1)],
                        start=(ki == 0), stop=(ki == NDT - 1))
                nc.scalar.copy(ssm_sb[:, LC * tch:LC * (tch + 1)], ps[0:96, :])

            ar_in = dram.tile([96, L], F32)
            ar_out = dram.tile([96, L], F32)
            nc.sync.dma_start(ar_in, ssm_sb)
            nc.gpsimd.collective_compute(
                "AllReduce", Op.add,
                replica_groups=[list(range(N_CORES))],
                ins=[ar_in.opt()], outs=[ar_out.opt()])
            ssmr_sb = state.tile([96, L], F32, name="ssmr_sb")
            nc.sync.dma_start(ssmr_sb, ar_out)
            nc.vector.tensor_copy(ssmr16, ssmr_sb)

            # ---------------- phase 3: dt = softplus(dt_proj @ dtr + b); dtx
            for mi in range(NDT):
                for tch in range(NCH):
                    ps = psum.tile([128, LC], F32, tag="mm", name="dt_ps")
                    nc.tensor.matmul(
                        ps, lhsT=dtwT_sb[:, mi, :],
                        rhs=ssmr16[0:DTR, LC * tch:LC * (tch + 1)],
                        start=True, stop=True)
                    # softplus(x+b) = ln(1 + e^(x+b)) via the exp/ln table set
                    spe = work.tile([128, LC], F32, tag="spe", bufs=2,
                                    name="spe")
                    nc.scalar.activation(spe, ps, Act.Exp,
                                         bias=dtb_sb[:, mi:mi + 1])
                    nc.gpsimd.tensor_scalar_add(spe, spe, 1.0)
                    nc.scalar.activation(
                        dt[mi][:, LC * tch:LC * (tch + 1)], spe, Act.Ln)
            for i in range(NDT):
                nc.vector.tensor_mul(dtx[i], dt[i], xc[i])

            # ---------------- phase 4: B_rep / C_rep (shared across d-tiles)
            for tch in range(NCH):
                for j, dest in ((0, brep), (1, crep)):
                    ps = psum.tile([128, LC], F32, tag="mm", name="bc_ps")
                    nc.tensor.matmul(ps, lhsT=selbc_sb[:, j, :],
                                     rhs=ssmr16[:, LC * tch:LC * (tch + 1)],
                                     start=True, stop=True)
                    nc.scalar.copy(dest[:, LC * tch:LC * (tch + 1)], ps)

            # ---------------- phase 5: the scan
            for g in range(NDT):
                for c in range(NCH):
                    tsl = slice(LC * c, LC * (c + 1))
                    yps = psum.tile([128, LC], F32, tag="yac", name="yps")
                    for li in range(TPG):
                        i = TPG * g + li
                        drep = psum.tile([128, LC], F32, tag="rep", bufs=4,
                                         name="drep")
                        nc.tensor.matmul(drep, lhsT=selrep_sb[:, li, :],
                                         rhs=dt[g][:, tsl],
                                         start=True, stop=True)
                        xrep = psum.tile([128, LC], F32, tag="rep", bufs=4,
                                         name="xrep")
                        nc.tensor.matmul(xrep, lhsT=selrep_sb[:, li, :],
                                         rhs=dtx[g][:, tsl],
                                         start=True, stop=True)
                        dA = work.tile([128, LC], F16, tag="dA")
                        exp_inst = nc.scalar.activation(
                            dA, drep, Act.Exp, scale=acols_sb[:, i:i + 1])
                        dBx = work.tile([128, LC], F16, tag="dBx")
                        dbx_inst = nc.vector.tensor_mul(dBx, xrep, brep[:, tsl])
                        # the scan's STT ISA struct has one sync-wait slot:
                        # make the DVE observe ACT here so the scan needs none
                        add_dep_helper(dbx_inst.ins, exp_inst.ins, sync=True,
                                       reason="absorb ACT wait ahead of scan")
                        hv = hbuf[g][:, li, :]
                        init = 0.0 if c == 0 else hv[:, LC - 1:LC]
                        nc.vector.tensor_tensor_scan(
                            hv, dA, dBx, init, op0=Op.mult, op1=Op.add)
                        hc = work.tile([128, LC], F16, tag="hc")
                        nc.vector.tensor_mul(hc, hv, crep[:, tsl])
                        nc.tensor.matmul(yps, lhsT=selmap_sb[:, li, :], rhs=hc,
                                         start=(li == 0), stop=(li == TPG - 1))
                    nc.scalar.copy(ysb[g][:, tsl], yps)

            # ---------------- phase 6: gating + out_proj
            for g in range(NDT):
                y2 = work.tile([128, L], F16, tag="y2")
                nc.vector.scalar_tensor_tensor(
                    y2, xc[g], dcol_sb[:, g:g + 1], ysb[g],
                    op0=Op.mult, op1=Op.add)
                nc.vector.tensor_mul(yg[g], y2, zsb[g])

            for tb in range(L // 128):
                for hch in range(H // LC):
                    ops = psum.tile([128, LC], F32, tag="mm", name="out_ps")
                    for g in range(NDT):
                        nc.tensor.matmul(
                            ops, lhsT=yg[g][:, 128 * tb:128 * (tb + 1)],
                            rhs=owT_sb[:, g, LC * hch:LC * (hch + 1)],
                            start=(g == 0), stop=(g == NDT - 1))
                    osb = work.tile([128, LC], F32, tag="osb")
                    nc.scalar.copy(osb, ops)
                    nc.sync.dma_start(
                        out[128 * tb:128 * (tb + 1), LC * hch:LC * (hch + 1)],
                        osb)
    _split_multi_waits(nc)
    return nc


_NC_CACHE = None


def _get_nc():
    global _NC_CACHE
    if _NC_CACHE is None:
        _NC_CACHE = _build()
    return _NC_CACHE


# ---------------------------------------------------------------- host side
def _make_in_maps(hidden_states, in_proj_w, conv_w, conv_b, x_proj_w,
                  dt_proj_w, dt_proj_b, A_log, D, out_proj_w):
    hsT16 = np.ascontiguousarray(hidden_states[0].T, dtype=np.float16)

    # selection matrices (shared by all cores)
    p = np.arange(128)
    li = np.arange(TPG)
    k = np.arange(128)
    # SelRep[k, li, p] = 1 iff k == 8*li + p//16
    selrep = (k[:, None, None] == 8 * li[None, :, None] +
              (p // 16)[None, None, :]).astype(np.float16)
    # Selmap[p, li, m] = 1 iff m == 8*li + p//16
    selmap = (k[None, None, :] == 8 * li[None, :, None] +
              (p // 16)[:, None, None]).astype(np.float16)
    k96 = np.arange(96)
    selb = (k96[:, None] == 64 + (p % 16)[None, :])
    selc = (k96[:, None] == 80 + (p % 16)[None, :])
    selbc = np.stack([selb, selc], axis=1).astype(np.float16)

    A = -np.exp(np.asarray(A_log, np.float64))     # [DI, DS]

    in_maps = []
    for c in range(N_CORES):
        s = slice(DIL * c, DIL * (c + 1))
        wxz = np.concatenate(
            [in_proj_w[s], in_proj_w[DI + DIL * c:DI + DIL * (c + 1)]], axis=0)
        Ac = A[s]                                   # [256, 16]
        ti = np.arange(NTILE)
        acols = Ac[8 * ti[None, :] + (p // 16)[:, None], (p % 16)[:, None]]
        in_maps.append({
            "hsT": hsT16,
            "wxzT": np.ascontiguousarray(wxz.T, dtype=np.float16),
            "owT": np.ascontiguousarray(out_proj_w[:, s].T, dtype=np.float16),
            "xpwT": np.ascontiguousarray(x_proj_w[:, s].T, dtype=np.float16),
            "dtwT": np.ascontiguousarray(dt_proj_w[s].T, dtype=np.float16),
            "selrep": selrep, "selmap": selmap, "selbc": selbc,
            "acols": np.ascontiguousarray(acols, np.float32),
            "convw": np.ascontiguousarray(
                conv_w[s, 0, :].reshape(NDT, 128, K).transpose(1, 0, 2),
                np.float32),
            "convb": np.ascontiguousarray(
                conv_b[s].reshape(NDT, 128).T, np.float32),
            "dtb": np.ascontiguousarray(
                dt_proj_b[s].reshape(NDT, 128).T, np.float32),
            "dcol": np.ascontiguousarray(
                D[s].reshape(NDT, 128).T, np.float32),
        })
    return in_maps


def kernel(hidden_states, in_proj_w, conv_w, conv_b, x_proj_w,
           dt_proj_w, dt_proj_b, A_log, D, out_proj_w):
    args = [np.asarray(a, np.float32) for a in
            (hidden_states, in_proj_w, conv_w, conv_b, x_proj_w,
             dt_proj_w, dt_proj_b, A_log, D, out_proj_w)]
    in_maps = _make_in_maps(*args)
    nc = _get_nc()
    res = run_bass_kernel_spmd(nc, in_maps, core_ids=list(range(N_CORES)))
    out = np.zeros((L, H), np.float64)
    for r in res.results:
        out += r["out"].astype(np.float64)
    return out.astype(np.float32).reshape(B, L, H)


# revision 18
# speedup vs baseline: 1.1508x; 1.1508x over previous
"""Trainium2 Bass kernel for CheemsMambaMixer (Mamba-1 selective SSM mixer).

Shapes: B=1, L=2048, H=1024, DI=2048, DS=16, DTR=64, K=4.
Sharding: tensor-parallel over the d_inner channel dim (256 channels/core on
8 cores).  The only cross-core communication is a [96, 2048] fp32 AllReduce
of the x_proj partial products; the out_proj row-parallel partials are summed
on the host.

Everything device-side runs in fp16 storage with fp32 accumulation (PSUM,
scan state), which lands ~1e-3 relative error vs the fp32 reference.
"""
import sys

sys.path.insert(0, "/opt/trn_rl_repo")

import numpy as np

import concourse.bass as bass
import concourse.tile as tile
from concourse import mybir
from concourse.bass_utils import run_bass_kernel_spmd
from concourse.tile_rust import add_dep_helper
import bass_rust as _bass_rust

# ---------------------------------------------------------------- constants
N_CORES = 8
B, L, H = 1, 2048, 1024
DI, DS, DTR, K = 2048, 16, 64, 4
DIL = DI // N_CORES          # 256 channels per core
NDT = DIL // 128             # 2 d-tiles of 128 channels
LC = 512                     # time chunk
NCH = L // LC                # 4 chunks
NTILE = DIL * DS // 128      # 32 (d,n)-tiles per core, 8 d x 16 n each
TPG = NTILE // NDT           # 16 tiles per d-group

F16 = mybir.dt.float16
F32 = mybir.dt.float32

N_PROCS = 27


class _SplitDrainTileContext(tile.TileContext):
    """Tail drain split into single-wait drains: the CTRL_NO ISA struct holds
    one sync-wait, but a kernel using all 8 HWDGE queues plus a collective
    accumulates 9+ outstanding procs at the tail."""

    def _drain_and_barrier(self, tick_clock, wait_clock):
        full = tick_clock.global_clock
        ticks = [(i, full.peek_next(i) - 1) for i in range(N_PROCS)]
        ticks = [(i, v) for i, v in ticks if v > 0]
        for i, v in ticks:
            c = _bass_rust.VectorClock()
            c.require_at_least(i, v)
            drain_inst = self.nc.sync.drain(fusable=False)
            wait_clock.add_sem_waits(
                drain_inst.ins, _bass_rust.ScopedClock({None: c}))
        self.nc.all_engine_barrier()
        assert self.sems is not None
        popped = self.nc._tile_sem_poison_stack.pop()
        assert popped is self._sem_poison
        self.nc.clear_and_free_semaphores(list(self.sems.allocated().values()))
        self.nc.all_engine_barrier()


def _split_multi_waits(nc):
    """TPB ISA structs carry a single sync-wait slot; Tile sometimes attaches
    several.  Hoist all but the last wait of every instruction onto dedicated
    single-wait NoOps on the same engine, inserted just before it."""
    wid = 0
    for bb in nc.main_func.blocks:
        insts = list(bb.instructions)
        out = []
        changed = False
        for ins in insts:
            si = ins.sync_info
            if si is not None and si.on_wait and len(si.on_wait) > 1:
                waits = list(si.on_wait)
                for w in waits[:-1]:
                    nop = _bass_rust.InstNoOp(name=f"W-split-{wid}", ins=[],
                                              outs=[])
                    wid += 1
                    nop.engine = ins.engine
                    nop.sync_info = mybir.SyncInfo(on_wait=[w], on_update=[])
                    out.append(nop)
                ins.sync_info = mybir.SyncInfo(on_wait=[waits[-1]],
                                               on_update=list(si.on_update or []))
                changed = True
            out.append(ins)
        if changed:
            bb.instructions = out


# ---------------------------------------------------------------- builder
def _build(single_core=False):
    nc = bass.Bass("TRN2", target_bir_lowering=False, debug=False,
                   num_devices=N_CORES)
    Act = mybir.ActivationFunctionType
    Op = mybir.AluOpType

    def din(name, shape, dtype=F16):
        return nc.dram_tensor(name, shape, dtype, kind="ExternalInput").ap()

    hsT = din("hsT", [H, L])                       # hidden_states[0].T
    wxzT = din("wxzT", [H, 2 * DIL])               # in_proj rows (x|z).T slice
    owT = din("owT", [DIL, H])                     # out_proj.T slice
    xpwT = din("xpwT", [DIL, 96])                  # x_proj.T slice
    dtwT = din("dtwT", [DTR, DIL])                 # dt_proj.T slice
    selrep = din("selrep", [128, TPG, 128])        # SelRep[k, li, p]
    selmap = din("selmap", [128, TPG, 128])        # Selmap[p, li, m]
    selbc = din("selbc", [96, 2, 128])             # SelB / SelC
    acols = din("acols", [128, NTILE], F32)        # A[d,n] per (tile, partition)
    convw = din("convw", [128, NDT, K], F32)
    convb = din("convb", [128, NDT], F32)
    dtb = din("dtb", [128, NDT], F32)
    dcol = din("dcol", [128, NDT], F32)
    out = nc.dram_tensor("out", [L, H], F32, kind="ExternalOutput").ap()

    with _SplitDrainTileContext(nc) as tc:
        import contextlib
        stack = contextlib.ExitStack()
        with stack:
            wpool = stack.enter_context(tc.tile_pool(name="wpool", bufs=1))
            state = stack.enter_context(tc.tile_pool(name="state", bufs=1))
            work = stack.enter_context(tc.tile_pool(name="work", bufs=3))
            psum = stack.enter_context(
                tc.tile_pool(name="psum", bufs=2, space="PSUM"))
            dram = stack.enter_context(
                tc.tile_pool(name="dram", bufs=1, space="DRAM"))

            # ---------------- load weights/constants
            wxzT_sb = wpool.tile([128, H // 128, 2 * DIL], F16)
            nc.sync.dma_start(wxzT_sb, wxzT.rearrange("(k p) m -> p k m", p=128))
            owT_sb = wpool.tile([128, NDT, H], F16)
            nc.sync.dma_start(owT_sb, owT.rearrange("(k p) h -> p k h", p=128))
            xpwT_sb = wpool.tile([128, NDT, 96], F16)
            nc.sync.dma_start(xpwT_sb, xpwT.rearrange("(k p) j -> p k j", p=128))
            dtwT_sb = wpool.tile([DTR, NDT, 128], F16)
            nc.sync.dma_start(dtwT_sb, dtwT.rearrange("k (m p) -> k m p", p=128))
            selrep_sb = wpool.tile([128, TPG, 128], F16)
            nc.sync.dma_start(selrep_sb, selrep)
            selmap_sb = wpool.tile([128, TPG, 128], F16)
            nc.sync.dma_start(selmap_sb, selmap)
            selbc_sb = wpool.tile([96, 2, 128], F16)
            nc.sync.dma_start(selbc_sb, selbc)
            acols_sb = wpool.tile([128, NTILE], F32)
            nc.sync.dma_start(acols_sb, acols)
            convw_sb = wpool.tile([128, NDT, K], F32)
            convw_dma = nc.sync.dma_start(convw_sb, convw)
            convb_sb = wpool.tile([128, NDT], F32)
            nc.sync.dma_start(convb_sb, convb)
            dtb_sb = wpool.tile([128, NDT], F32)
            nc.sync.dma_start(dtb_sb, dtb)
            dcol_sb = wpool.tile([128, NDT], F32)
            nc.sync.dma_start(dcol_sb, dcol)

            # Wait-slot fencing: TensorScalarPtr-class DVE ops (tensor_scalar,
            # scalar_tensor_tensor, tensor_tensor_scan) have very few sync-wait
            # slots in their ISA structs.  A tiny TensorTensor op (2 wait
            # slots) placed just before makes the in-order DVE observe the
            # producers' semaphores so the fragile op needs no new waits.
            fence_scratch = wpool.tile([128, 4], F32)

            def dve_observe(*insts):
                insts = [i for i in insts if i is not None]
                for j in range(0, len(insts), 1):
                    f = nc.vector.tensor_mul(
                        fence_scratch[:, 0:1], fence_scratch[:, 0:1],
                        fence_scratch[:, 0:1])
                    for d in insts[j:j + 1]:
                        add_dep_helper(f.ins, d.ins, sync=True,
                                       reason="dve wait fence")

            # The ACT engine loads ONE spline-table set; none contains
            # exp+silu+softplus together.  natural_log_exp_and_others has
            # {exp, ln, copy, identity}, so silu and softplus are synthesized:
            #   softplus(x) = ln(1 + e^x)
            #   silu(v) = v * sigma(v),  sigma(v) = exp(-ln(1 + e^-v))
            def silu_into(dst, v, pool):
                t1 = pool.tile([128, L], F32, tag="silu_t1", name="t1", bufs=1)
                nc.scalar.activation(t1, v, Act.Exp, scale=-1.0)
                nc.gpsimd.tensor_scalar_add(t1, t1, 1.0)
                t2 = pool.tile([128, L], F32, tag="silu_t2", name="t2", bufs=1)
                nc.scalar.activation(t2, t1, Act.Ln)
                t3 = pool.tile([128, L], F16, tag="silu_t3", name="t3", bufs=1)
                nc.scalar.activation(t3, t2, Act.Exp, scale=-1.0)
                nc.vector.tensor_mul(dst, v, t3)

            # persistent state tensors
            xc = [state.tile([128, L], F16, name=f"xc{i}") for i in range(NDT)]
            zsb = [state.tile([128, L], F16, name=f"zsb{i}") for i in range(NDT)]
            dt = [state.tile([128, L], F16, name=f"dt{i}") for i in range(NDT)]
            dtx = [state.tile([128, L], F16, name=f"dtx{i}") for i in range(NDT)]
            ssm_sb = state.tile([96, L], F32, name="ssm_sb")
            ssmr16 = state.tile([96, L], F16, name="ssmr16")

            # ---------------- phase 1: in_proj + conv + silu
            with tc.tile_pool(name="inproj", bufs=1) as inproj:
                hsT_sb = inproj.tile([128, H // 128, L], F16)
                nc.sync.dma_start(hsT_sb, hsT.rearrange("(k p) t -> p k t", p=128))
                xpad = [inproj.tile([128, K - 1 + L], F16, name=f"xpad{i}")
                        for i in range(NDT)]
                xpad_evacs = [[] for _ in range(NDT)]
                for i in range(NDT):
                    nc.vector.memset(xpad[i][:, 0:K - 1], 0.0)

                for dm in range(2 * NDT):
                    for tch in range(NCH):
                        ps = psum.tile([128, LC], F32, tag="mm")
                        for k in range(H // 128):
                            nc.tensor.matmul(
                                ps,
                                lhsT=wxzT_sb[:, k, 128 * dm:128 * (dm + 1)],
                                rhs=hsT_sb[:, k, LC * tch:LC * (tch + 1)],
                                start=(k == 0), stop=(k == H // 128 - 1))
                        if dm < NDT:  # x branch -> conv input
                            ev = nc.scalar.copy(
                                xpad[dm][:, K - 1 + LC * tch:K - 1 + LC * (tch + 1)],
                                ps)
                            xpad_evacs[dm].append(ev)
                        else:         # z branch -> SBUF, silu later
                            nc.scalar.copy(
                                zsb[dm - NDT][:, LC * tch:LC * (tch + 1)], ps)

                for i in range(NDT):
                    silu_into(zsb[i], zsb[i], inproj)

                # causal depthwise conv (K=4) + bias + silu
                for i in range(NDT):
                    acc = inproj.tile([128, L], F16, tag="convacc", bufs=2, name="acc")
                    dve_observe(*xpad_evacs[i], convw_dma)
                    nc.vector.tensor_scalar(
                        acc, xpad[i][:, 0:L], convw_sb[:, i, 0:1], None,
                        op0=Op.mult)
                    for k in range(1, K):
                        nc.vector.scalar_tensor_tensor(
                            acc, xpad[i][:, k:k + L], convw_sb[:, i, k:k + 1],
                            acc, op0=Op.mult, op1=Op.add)
                    nc.vector.tensor_scalar_add(acc, acc, convb_sb[:, i:i + 1])
                    silu_into(xc[i], acc, inproj)

            # scan-phase persistents allocated after inproj released its zone
            scanp = stack.enter_context(tc.tile_pool(name="scanp", bufs=1))
            brep = scanp.tile([128, L], F16, name="brep")
            crep = scanp.tile([128, L], F16, name="crep")
            ysb = [scanp.tile([128, L], F16, name=f"ysb{g}") for g in range(NDT)]
            yg = [scanp.tile([128, L], F16, name=f"yg{g}") for g in range(NDT)]

            # ---------------- phase 2: x_proj partial + AllReduce
            for tch in range(NCH):
                ps = psum.tile([128, LC], F32, tag="mm", name="ssm_ps")
                for ki in range(NDT):
                    nc.tensor.matmul(
                        ps[0:96, :], lhsT=xpwT_sb[:, ki, :],
                        rhs=xc[ki][:, LC * tch:LC * (tch + 1)],
                        start=(ki == 0), stop=(ki == NDT - 1))
                nc.scalar.copy(ssm_sb[:, LC * tch:LC * (tch + 1)], ps[0:96, :])

            ar_in = dram.tile([96, L], F32)
            ar_out = dram.tile([96, L], F32)
            nc.sync.dma_start(ar_in, ssm_sb)
            if single_core:
                nc.sync.dma_start(ar_out, ar_in)
            else:
                nc.gpsimd.collective_compute(
                    "AllReduce", Op.add,
                    replica_groups=[list(range(N_CORES))],
                    ins=[ar_in.opt()], outs=[ar_out.opt()])
            ssmr_sb = state.tile([96, L], F32, name="ssmr_sb")
            nc.sync.dma_start(ssmr_sb, ar_out)
            nc.vector.tensor_copy(ssmr16, ssmr_sb)

            # ---------------- phase 3: dt = softplus(dt_proj @ dtr + b); dtx
            for mi in range(NDT):
                for tch in range(NCH):
                    ps = psum.tile([128, LC], F32, tag="mm", name="dt_ps")
                    nc.tensor.matmul(
                        ps, lhsT=dtwT_sb[:, mi, :],
                        rhs=ssmr16[0:DTR, LC * tch:LC * (tch + 1)],
                        start=True, stop=True)
                    # softplus(x+b) = ln(1 + e^(x+b)) via the exp/ln table set
                    spe = work.tile([128, LC], F32, tag="spe", bufs=2,
                                    name="spe")
                    nc.scalar.activation(spe, ps, Act.Exp,
                                         bias=dtb_sb[:, mi:mi + 1])
                    nc.gpsimd.tensor_scalar_add(spe, spe, 1.0)
                    nc.scalar.activation(
                        dt[mi][:, LC * tch:LC * (tch + 1)], spe, Act.Ln)
            for i in range(NDT):
                nc.vector.tensor_mul(dtx[i], dt[i], xc[i])

            # ---------------- phase 4: B_rep / C_rep (shared across d-tiles)
            for tch in range(NCH):
                for j, dest in ((0, brep), (1, crep)):
                    ps = psum.tile([128, LC], F32, tag="mm", name="bc_ps")
                    nc.tensor.matmul(ps, lhsT=selbc_sb[:, j, :],
                                     rhs=ssmr16[:, LC * tch:LC * (tch + 1)],
                                     start=True, stop=True)
                    nc.scalar.copy(dest[:, LC * tch:LC * (tch + 1)], ps)

            # ---------------- phase 5: the scan
            # Full-L scans: per (d,n)-tile, build dA/dBx for all 2048 steps,
            # run one tensor_tensor_scan, multiply by C on GPSIMD, and reduce
            # n via selection matmuls accumulating dense y per chunk bank.
            for g in range(NDT):
                yac = [psum.tile([128, LC], F32, tag="yac", bufs=4,
                                 name=f"yac{c}") for c in range(NCH)]
                for li in range(TPG):
                    i = TPG * g + li
                    dA = work.tile([128, L], F16, tag="dA", bufs=2)
                    last_exp = None
                    for c in range(NCH):
                        tsl = slice(LC * c, LC * (c + 1))
                        drep = psum.tile([128, LC], F32, tag="mm", name="drep")
                        nc.tensor.matmul(drep, lhsT=selrep_sb[:, li, :],
                                         rhs=dt[g][:, tsl],
                                         start=True, stop=True)
                        last_exp = nc.scalar.activation(
                            dA[:, tsl], drep, Act.Exp,
                            scale=acols_sb[:, i:i + 1])
                    dBx = work.tile([128, L], F16, tag="dBx", bufs=2)
                    for half in range(2):
                        hsl = slice(1024 * half, 1024 * (half + 1))
                        dxp = psum.tile([128, 1024], F32, tag="dxrep", bufs=1,
                                        name="dxp")
                        for cc in range(2):
                            nc.tensor.matmul(
                                dxp[:, LC * cc:LC * (cc + 1)],
                                lhsT=selrep_sb[:, li, :],
                                rhs=dtx[g][:, 1024 * half + LC * cc:
                                           1024 * half + LC * (cc + 1)],
                                start=True, stop=True)
                        dbx_inst = nc.vector.tensor_mul(
                            dBx[:, hsl], dxp, brep[:, hsl])
                        if half == 0:
                            add_dep_helper(dbx_inst.ins, last_exp.ins,
                                           sync=True,
                                           reason="absorb ACT wait for scan")
                    hv = work.tile([128, L], F16, tag="hv", bufs=2)
                    nc.vector.tensor_tensor_scan(
                        hv, dA, dBx, 0.0, op0=Op.mult, op1=Op.add)
                    hc = work.tile([128, L], F16, tag="hc", bufs=2)
                    nc.gpsimd.tensor_mul(hc, hv, crep)
                    for c in range(NCH):
                        tsl = slice(LC * c, LC * (c + 1))
                        nc.tensor.matmul(yac[c], lhsT=selmap_sb[:, li, :],
                                         rhs=hc[:, tsl],
                                         start=(li == 0), stop=(li == TPG - 1))
                for c in range(NCH):
                    nc.scalar.copy(ysb[g][:, LC * c:LC * (c + 1)], yac[c])

            # ---------------- phase 6: gating + out_proj
            for g in range(NDT):
                y2 = work.tile([128, L], F16, tag="y2", bufs=1)
                nc.vector.scalar_tensor_tensor(
                    y2, xc[g], dcol_sb[:, g:g + 1], ysb[g],
                    op0=Op.mult, op1=Op.add)
                nc.vector.tensor_mul(yg[g], y2, zsb[g])

            for tb in range(L // 128):
                for hch in range(H // LC):
                    ops = psum.tile([128, LC], F32, tag="mm", name="out_ps")
                    for g in range(NDT):
                        nc.tensor.matmul(
                            ops, lhsT=yg[g][:, 128 * tb:128 * (tb + 1)],
                            rhs=owT_sb[:, g, LC * hch:LC * (hch + 1)],
                            start=(g == 0), stop=(g == NDT - 1))
                    osb = work.tile([128, LC], F32, tag="osb")
                    nc.scalar.copy(osb, ops)
                    nc.sync.dma_start(
                        out[128 * tb:128 * (tb + 1), LC * hch:LC * (hch + 1)],
                        osb)
    _split_multi_waits(nc)
    return nc


_NC_CACHE = None


def _get_nc():
    global _NC_CACHE
    if _NC_CACHE is None:
        _NC_CACHE = _build()
    return _NC_CACHE


# ---------------------------------------------------------------- host side
def _make_in_maps(hidden_states, in_proj_w, conv_w, conv_b, x_proj_w,
                  dt_proj_w, dt_proj_b, A_log, D, out_proj_w):
    hsT16 = np.ascontiguousarray(hidden_states[0].T, dtype=np.float16)

    # selection matrices (shared by all cores)
    p = np.arange(128)
    li = np.arange(TPG)
    k = np.arange(128)
    # SelRep[k, li, p] = 1 iff k == 8*li + p//16
    selrep = (k[:, None, None] == 8 * li[None, :, None] +
              (p // 16)[None, None, :]).astype(np.float16)
    # Selmap[p, li, m] = 1 iff m == 8*li + p//16
    selmap = (k[None, None, :] == 8 * li[None, :, None] +
              (p // 16)[:, None, None]).astype(np.float16)
    k96 = np.arange(96)
    selb = (k96[:, None] == 64 + (p % 16)[None, :])
    selc = (k96[:, None] == 80 + (p % 16)[None, :])
    selbc = np.stack([selb, selc], axis=1).astype(np.float16)

    A = -np.exp(np.asarray(A_log, np.float64))     # [DI, DS]

    in_maps = []
    for c in range(N_CORES):
        s = slice(DIL * c, DIL * (c + 1))
        wxz = np.concatenate(
            [in_proj_w[s], in_proj_w[DI + DIL * c:DI + DIL * (c + 1)]], axis=0)
        Ac = A[s]                                   # [256, 16]
        ti = np.arange(NTILE)
        acols = Ac[8 * ti[None, :] + (p // 16)[:, None], (p % 16)[:, None]]
        in_maps.append({
            "hsT": hsT16,
            "wxzT": np.ascontiguousarray(wxz.T, dtype=np.float16),
            "owT": np.ascontiguousarray(out_proj_w[:, s].T, dtype=np.float16),
            "xpwT": np.ascontiguousarray(x_proj_w[:, s].T, dtype=np.float16),
            "dtwT": np.ascontiguousarray(dt_proj_w[s].T, dtype=np.float16),
            "selrep": selrep, "selmap": selmap, "selbc": selbc,
            "acols": np.ascontiguousarray(acols, np.float32),
            "convw": np.ascontiguousarray(
                conv_w[s, 0, :].reshape(NDT, 128, K).transpose(1, 0, 2),
                np.float32),
            "convb": np.ascontiguousarray(
                conv_b[s].reshape(NDT, 128).T, np.float32),
            "dtb": np.ascontiguousarray(
                dt_proj_b[s].reshape(NDT, 128).T, np.float32),
            "dcol": np.ascontiguousarray(
                D[s].reshape(NDT, 128).T, np.float32),
        })
    return in_maps


def kernel(hidden_states, in_proj_w, conv_w, conv_b, x_proj_w,
           dt_proj_w, dt_proj_b, A_log, D, out_proj_w):
    args = [np.asarray(a, np.float32) for a in
            (hidden_states, in_proj_w, conv_w, conv_b, x_proj_w,
             dt_proj_w, dt_proj_b, A_log, D, out_proj_w)]
    in_maps = _make_in_maps(*args)
    nc = _get_nc()
    res = run_bass_kernel_spmd(nc, in_maps, core_ids=list(range(N_CORES)))
    out = np.zeros((L, H), np.float64)
    for r in res.results:
        out += r["out"].astype(np.float64)
    return out.astype(np.float32).reshape(B, L, H)
